# revision 1
# baseline (speedup 1.0000x reference)
"""TRN2 Bass/Tile kernel for nn_MultiHeadSelfAttention (heads-axis attention
variant + output projection), data-parallel over 8 NeuronCores.

Math per position p (of N*S=16384):
  A = softmax_j(Q[p] @ K[p].T / sqrt(D)) with mask     (Q[p],K[p]: [H=32, D=128])
  X[p] = vec(A @ V[p])                                 ([E=4096])
  Y[p] = X[p] @ W_out.T + b_out

Sharding: each core takes 2048 consecutive positions (data-parallel; no
collectives). W_out is replicated. Inside a core:
  - scores^T per 4-position group via one 128x128 PE matmul (block-diagonal
    valid, off-blocks masked to 0 in exp domain)
  - exp on ACT, mask multiply + softmax normalization on DVE, denominator via
    PE matmul against a ones column
  - PE transpose to head-major layout, then the 2048x4096x4096 output
  - projection as bf16 PE matmuls (fp32 accumulation), streaming W column
    chunks

Host-side packing only reshapes/casts inputs (bf16) - all FLOPs run on device.
"""
import os
import sys

for _p in ('/opt/trn_rl_repo',):
    if _p not in sys.path and os.path.isdir(_p):
        sys.path.insert(0, _p)

from contextlib import ExitStack

import numpy as np
import ml_dtypes

import concourse.bass as bass
import concourse.mybir as mybir
import concourse.tile as tile
from concourse.masks import make_identity
from concourse.bass_utils import run_bass_kernel_spmd

F32 = mybir.dt.float32
BF16 = mybir.dt.bfloat16
EXP = mybir.ActivationFunctionType.Exp
COPY = mybir.ActivationFunctionType.Copy

N, S, E, H, D = 4, 4096, 4096, 32, 128
NCORES = 8
T = (N * S) // NCORES      # positions per core = 2048
NQ = T // 16               # quads (16 positions) per core = 128
NST = 4                    # super-tiles per core (512 positions each)
QPS = NQ // NST            # quads per super-tile = 32

LAST_RESULT = None         # BassKernelResults of the most recent run


# ───────────────────────── walrus wait-count workaround ─────────────────────
def _split_waits_json_bytes(raw: bytes):
    """The walrus build in this container accepts at most ONE sync wait per
    instruction; hoist extra waits onto standalone EventSemaphore
    instructions on the same engine immediately before the instruction."""
    import orjson
    d = orjson.loads(raw)
    ctr = [0]

    def fix_block(blk):
        insts = blk.get("instructions")
        if not insts:
            return
        out = []
        for inst in insts:
            si = inst.get("sync_info")
            waits = si.get("on_wait") if si else None
            if waits and len(waits) > 1:
                for w in waits[:-1]:
                    ctr[0] += 1
                    out.append({
                        "name": f"I-wsplit-{ctr[0]}",
                        "engine": inst.get("engine", "SP"),
                        "opcode": "EventSemaphore",
                        "ins": [], "outs": [],
                        "sync_info": {"on_update": [], "on_wait": [w]},
                    })
                si["on_wait"] = [waits[-1]]
            out.append(inst)
        blk["instructions"] = out

    def walk(o):
        if isinstance(o, dict):
            if "instructions" in o:
                fix_block(o)
            for v in o.values():
                walk(v)
        elif isinstance(o, list):
            for v in o:
                walk(v)
    walk(d)
    return orjson.dumps(d)


def _patch_nc(nc):
    orig = nc.to_json_bytes
    nc.to_json_bytes = lambda: _split_waits_json_bytes(orig())
    return nc


# ───────────────────────────── program builder ──────────────────────────────
def build_nc(nst=NST, qps=QPS, neoc=8):
    """Per-core Bass program. Input layouts (t_pos = nst*qps*16 positions,
    pos(q,g,p) = q*16 + g*4 + p):
      qt [nq,128,512] bf16: qt[q][d][g*128+p*32+h] = Q[pos,h,d]/sqrt(D)
      kt [nq,128,512] bf16: kt[q][d][g*128+p*32+j] = K[pos,j,d]
      v  [nq,128,512] bf16: v[q][p*32+j][g*128+d]  = V[pos,j,d]
      wt [32,128,4096] bf16: wt[ko][kd][eo] = W_out[eo, ko*128+kd]
      bbc [128,4096] f32 (bias, partition-broadcast); em [128,128] f32 mask
      out [t_pos,4096] f32
    """
    nq = nst * qps
    t_pos = nq * 16
    nc = bass.Bass()
    qt_h = nc.dram_tensor("qt", [nq, 128, 512], BF16, kind="ExternalInput")
    kt_h = nc.dram_tensor("kt", [nq, 128, 512], BF16, kind="ExternalInput")
    v_h = nc.dram_tensor("v", [nq, 128, 512], BF16, kind="ExternalInput")
    wt_h = nc.dram_tensor("wt", [32, 128, 4096], BF16, kind="ExternalInput")
    bbc_h = nc.dram_tensor("bbc", [128, 4096], F32, kind="ExternalInput")
    em_h = nc.dram_tensor("em", [128, 128], F32, kind="ExternalInput")
    out_h = nc.dram_tensor("out", [t_pos, 4096], F32, kind="ExternalOutput")

    with tile.TileContext(nc) as tc, ExitStack() as ctx:
        const = ctx.enter_context(tc.tile_pool(name="const", bufs=1))
        ones_t = const.tile([128, 1], BF16, tag="ones")
        nc.gpsimd.memset(ones_t[:], 1.0)
        ident = const.tile([128, 128], BF16, tag="ident")
        make_identity(nc, ident[:])
        em_sb = const.tile([128, 128], F32, tag="em")
        nc.sync.dma_start(em_sb[:], em_h[:])
        b_sb = const.tile([128, 4096], F32, tag="bias")
        nc.sync.dma_start(b_sb[:], bbc_h[:])

        qt_pool = ctx.enter_context(tc.tile_pool(name="qt", bufs=3))
        kt_pool = ctx.enter_context(tc.tile_pool(name="kt", bufs=3))
        v_pool = ctx.enter_context(tc.tile_pool(name="v", bufs=3))
        et_pool = ctx.enter_context(tc.tile_pool(name="et", bufs=2))
        etm_pool = ctx.enter_context(tc.tile_pool(name="etm", bufs=2))
        zr_pool = ctx.enter_context(tc.tile_pool(name="zr", bufs=2))
        un_pool = ctx.enter_context(tc.tile_pool(name="un", bufs=2))
        xt_pool = ctx.enter_context(tc.tile_pool(name="xt", bufs=2))
        wt_pool = ctx.enter_context(tc.tile_pool(name="wt", bufs=2))
        os_pool = ctx.enter_context(tc.tile_pool(name="os", bufs=4))

        st_psum = ctx.enter_context(tc.tile_pool(name="stp", bufs=1, space="PSUM"))
        up_psum = ctx.enter_context(tc.tile_pool(name="upp", bufs=1, space="PSUM"))
        z_psum = ctx.enter_context(tc.tile_pool(name="zp", bufs=1, space="PSUM"))
        t_psum = ctx.enter_context(tc.tile_pool(name="tp", bufs=1, space="PSUM"))
        pp_psum = ctx.enter_context(tc.tile_pool(name="pp", bufs=4, space="PSUM"))

        ntile = qps * 16 // 128  # 128-position tiles per super-tile
        assert qps % 8 == 0 and ntile * 128 == qps * 16

        for stt in range(nst):
            xt = xt_pool.tile([128, ntile * 4096], BF16)
            # xt col: tloc*4096 + h*128 + (q%8)*16 + g*4 + p
            for q in range(qps):
                Q = stt * qps + q
                qt_sb = qt_pool.tile([128, 512], BF16)
                nc.sync.dma_start(qt_sb[:], qt_h[Q, :, :])
                kt_sb = kt_pool.tile([128, 512], BF16)
                nc.sync.dma_start(kt_sb[:], kt_h[Q, :, :])
                v_sb = v_pool.tile([128, 512], BF16)
                nc.sync.dma_start(v_sb[:], v_h[Q, :, :])

                # scores^T per group: [(p,j), (p,h)] = K4t.T @ Q4t
                stp = st_psum.tile([128, 512], F32)
                for g in range(4):
                    s = slice(g * 128, (g + 1) * 128)
                    nc.tensor.matmul(stp[:, s], lhsT=kt_sb[:, s], rhs=qt_sb[:, s])

                et = et_pool.tile([128, 512], F32)
                nc.scalar.activation(et[:], stp[:], EXP)

                # zero off-diagonal blocks / apply user mask (exp domain)
                etm = etm_pool.tile([128, 512], BF16)
                nc.vector.tensor_mul(
                    etm[:].rearrange("part (g c) -> part g c", g=4),
                    et[:].rearrange("part (g c) -> part g c", g=4),
                    em_sb[:].unsqueeze(1).broadcast_to([128, 4, 128]),
                )

                # U' = E^T V (unnormalized), Z via ones column
                upp = up_psum.tile([128, 512], F32)
                zp = z_psum.tile([128, 4], F32)
                for g in range(4):
                    s = slice(g * 128, (g + 1) * 128)
                    nc.tensor.matmul(upp[:, s], lhsT=etm[:, s], rhs=v_sb[:, s])
                    nc.tensor.matmul(zp[:, g:g + 1], lhsT=etm[:, s], rhs=ones_t[:])

                zr = zr_pool.tile([128, 4], F32)
                nc.vector.reciprocal(zr[:], zp[:])

                un = un_pool.tile([128, 512], BF16)
                nc.vector.tensor_mul(
                    un[:].rearrange("part (g d) -> part g d", g=4),
                    upp[:].rearrange("part (g d) -> part g d", g=4),
                    zr[:].unsqueeze(2).broadcast_to([128, 4, 128]),
                )

                # transpose to [d, (p,h)] and scatter into head-major xt
                tp = t_psum.tile([128, 512], BF16)
                for g in range(4):
                    s = slice(g * 128, (g + 1) * 128)
                    nc.tensor.transpose(tp[:, s], un[:, s], ident[:])

                tloc, qm8 = q // 8, q % 8
                dst = (xt[:]
                       .rearrange("part (t h q g p) -> part t h q g p",
                                  t=ntile, h=32, q=8, g=4)
                       [:, tloc, :, qm8, :, :])
                src = tp[:].rearrange("part (g p h) -> part h g p", g=4, p=4)
                nc.scalar.activation(dst, src, COPY)

            # projection for this super-tile: Y = X @ W^T + b
            for eoc in range(neoc):
                wts = wt_pool.tile([128, 16384], BF16)
                nc.sync.dma_start(
                    wts[:].rearrange("part (ko eo) -> part ko eo", ko=32),
                    wt_h[:, :, eoc * 512:(eoc + 1) * 512]
                    .rearrange("ko kd eo -> kd ko eo"),
                )
                for t in range(ntile):
                    pp = pp_psum.tile([128, 512], F32)
                    for ko in range(32):
                        nc.tensor.matmul(
                            pp[:],
                            lhsT=xt[:, t * 4096 + ko * 128: t * 4096 + (ko + 1) * 128],
                            rhs=wts[:, ko * 512:(ko + 1) * 512],
                            start=(ko == 0), stop=(ko == 31),
                        )
                    os_sb = os_pool.tile([128, 512], F32)
                    nc.vector.tensor_add(os_sb[:], pp[:],
                                         b_sb[:, eoc * 512:(eoc + 1) * 512])
                    nc.sync.dma_start(
                        out_h[stt * qps * 16 + t * 128: stt * qps * 16 + (t + 1) * 128,
                              eoc * 512:(eoc + 1) * 512],
                        os_sb[:],
                    )
    _patch_nc(nc)
    return nc


# ─────────────────────────────── host packing ───────────────────────────────
def _pack_core(q2d, k2d, v2d, nq):
    scale = np.float32(1.0 / np.sqrt(D))
    bf = ml_dtypes.bfloat16
    q5 = (q2d * scale).reshape(nq, 4, 4, 32, 128)            # q g p h d
    qt = np.ascontiguousarray(q5.transpose(0, 4, 1, 2, 3)).reshape(nq, 128, 512).astype(bf)
    k5 = k2d.reshape(nq, 4, 4, 32, 128)
    kt = np.ascontiguousarray(k5.transpose(0, 4, 1, 2, 3)).reshape(nq, 128, 512).astype(bf)
    v5 = v2d.reshape(nq, 4, 4, 32, 128)                      # q g p j d
    vv = np.ascontiguousarray(v5.transpose(0, 2, 3, 1, 4)).reshape(nq, 128, 512).astype(bf)
    return qt, kt, vv


def _pack_em(mask_hj):
    em = np.zeros((128, 128), dtype=np.float32)
    m = mask_hj.astype(np.float32)          # [h, j]; 0 -> drop, else keep
    m = (m != 0).astype(np.float32)
    for p in range(4):
        em[p * 32:(p + 1) * 32, p * 32:(p + 1) * 32] = m.T
    return em


_NC_CACHE = {}


def kernel(values, keys, queries, mask, W_out, b_out):
    global LAST_RESULT
    values = np.asarray(values, dtype=np.float32)
    keys = np.asarray(keys, dtype=np.float32)
    queries = np.asarray(queries, dtype=np.float32)
    mask = np.asarray(mask)
    W_out = np.asarray(W_out, dtype=np.float32)
    b_out = np.asarray(b_out, dtype=np.float32)

    if 'full' not in _NC_CACHE:
        _NC_CACHE['full'] = build_nc()
    nc = _NC_CACHE['full']

    bf = ml_dtypes.bfloat16
    wt = np.ascontiguousarray(W_out.T).reshape(32, 128, 4096).astype(bf)
    bbc = np.ascontiguousarray(np.broadcast_to(b_out, (128, 4096)))

    q_all = queries.reshape(N * S, E)
    k_all = keys.reshape(N * S, E)
    v_all = values.reshape(N * S, E)

    in_maps = []
    for c in range(NCORES):
        sl = slice(c * T, (c + 1) * T)
        qt, kt, vv = _pack_core(q_all[sl], k_all[sl], v_all[sl], NQ)
        em = _pack_em(mask[c * T // S, 0])
        in_maps.append({"qt": qt, "kt": kt, "v": vv, "wt": wt,
                        "bbc": bbc, "em": em})

    trace = os.environ.get("MHA_TRACE") == "1"
    kwargs = {}
    if trace:
        _install_ntff_hook()
        kwargs = dict(trace=True)
        import tempfile
        kwargs["tmpdir"] = os.environ.get("MHA_TRACE_DIR") or tempfile.mkdtemp()

    res = run_bass_kernel_spmd(nc, in_maps, list(range(NCORES)), **kwargs)
    LAST_RESULT = res
    out = np.concatenate([res.results[c]["out"] for c in range(NCORES)], axis=0)
    return out.reshape(N, S, E).astype(np.float32)


# ──────────────── NTFF profile hook (tracing only; optional) ────────────────
def _install_ntff_hook():
    import contextlib, ctypes, types
    if 'antenv.axon_hooks' in sys.modules:
        return
    so_path = '/opt/axon/libaxon_pjrt.so'
    if not os.path.exists(so_path):
        return
    lib = ctypes.CDLL(so_path)
    if not hasattr(lib, 'axon_start_nrt_profile'):
        return
    lib.axon_start_nrt_profile.argtypes = [ctypes.POINTER(ctypes.c_int64), ctypes.c_size_t]
    lib.axon_start_nrt_profile.restype = ctypes.c_int64
    lib.axon_stop_nrt_profile.argtypes = [ctypes.c_char_p]
    lib.axon_stop_nrt_profile.restype = ctypes.c_int64

    @contextlib.contextmanager
    def _hook(output_dir, device_ids):
        import jax
        jax.devices()
        if device_ids:
            ids = (ctypes.c_int64 * len(device_ids))(*device_ids)
            rc = lib.axon_start_nrt_profile(ids, len(device_ids))
        else:
            rc = lib.axon_start_nrt_profile(None, 0)
        if rc != 0:
            raise RuntimeError(f"axon_start_nrt_profile rc={rc}")
        try:
            yield
        finally:
            n = lib.axon_stop_nrt_profile(str(output_dir).encode())
            print(f"profile: {n} file(s) written to {output_dir}", file=sys.stderr)

    mod = types.ModuleType('antenv.axon_hooks')
    mod.get_axon_ntff_profile_hook = lambda: _hook
    mod.set_axon_ntff_profile_hook = lambda h: None
    sys.modules['antenv.axon_hooks'] = mod
    import antenv
    antenv.axon_hooks = mod
